# revision 59
# baseline (speedup 1.0000x reference)
"""Multi-head causal attention (B=2, S=2048, D=1024, H=16) on 8 trn2 NeuronCores.

Sharding: data-parallel over batch (2) x tensor-parallel over heads (4 groups of
4 heads).  Core c = 4*b + g handles batch b, heads [4g, 4g+4).  Each core
computes a partial output  ctx_g @ Wo_g.T  [2048, 1024]; the host sums the 4
partials per batch.

Within-core dataflow (v2):
  qT,kT = W @ X.T        float32r matmuls; Wq pre-scaled by sqrt(dk)=8 on host
                         so scores come out of the PE already scaled
  v     = X @ Wv.T       natural [s, dv], stored bf16
  S     = qT.T @ kT      per 128-row query tile, contiguous 2-bank PSUM regions
                         (<=1024 wide); causal mask + padding written by a PE
                         matmul (ident x mask-tile) accumulated under the
                         diagonal score matmul; all score matmuls >=256 moving
  m     = rowmax         ONE negated reduce_max per region (DVE) -> exp bias
  P     = exp(S - m)     ONE exp per region on ACT, accum_out -> Z
  P    *= 1/Z            one DVE 4x bf16 pass per tile
  P_T   = PE transpose   bf16 128x128 blocks into 1024-wide dual-head tiles,
                         one PSUM->SBUF copy per k-chunk (split DVE/ACT)
  ctxT  = v.T @ P_T      bf16, two heads col-packed, accumulated over k tiles
  out  += ctxT.T @ WoT   bf16 ctx x bf16 Wo per s-tile; out DMAs ride the
                         idle Pool queue

Projection chunks are interleaved with attention rounds (round u needs input
chunks 0..u only) and scores of group g+1 overlap the transpose/PV tail of
group g.  _DMA_TR gates a dormant XBAR-transpose path for the widest tiles
(correct source ordering needs ACT-side normalize + ACT-issued DMA; blocked
on SBUF for the no-reuse PT pool — see project notes).
"""

import numpy as np

B, S, D, H = 2, 2048, 1024, 16
DK = D // H          # 64
JC = 256             # per-core projection width (4 heads * 64)
NQT = S // 128       # 16 query tiles
NU4 = S // 512       # 4 query supertiles
_SCALE = float(DK) ** 0.5  # 8.0  (folded into Wq on the host)
_MASKVAL = -1.0e30

_cached = {}

# knobs
_DMA_TR = False
_DBG_PTT = False
_PTSB_PAT = (0, 1)   # ptsb copy engine pattern: 0=DVE, 1=ACT
_PSS_BUFS = 2
_PT_BUFS = 18


def _build_nc(reps=1):
    from contextlib import ExitStack

    import concourse.mybir as mybir
    import concourse.tile as tile
    from concourse import bacc

    F32 = mybir.dt.float32
    F32R = mybir.dt.float32r
    BF16 = mybir.dt.bfloat16
    EXP = mybir.ActivationFunctionType.Exp
    COPY = mybir.ActivationFunctionType.Copy
    AX = mybir.AxisListType.X
    MIN = mybir.AluOpType.min

    nc = bacc.Bacc("TRN2", target_bir_lowering=False)

    xtq_d = nc.dram_tensor("xtq", [D, S], F32R, kind="ExternalInput")
    xtk_d = nc.dram_tensor("xtk", [D, S], F32R, kind="ExternalInput")
    xtv_d = nc.dram_tensor("xtv", [D, S], F32R, kind="ExternalInput")
    wqt_d = nc.dram_tensor("wqt", [D, JC], F32R, kind="ExternalInput")
    wkt_d = nc.dram_tensor("wkt", [D, JC], F32R, kind="ExternalInput")
    wvt_d = nc.dram_tensor("wvt", [D, JC], F32R, kind="ExternalInput")
    wot_d = nc.dram_tensor("wot", [JC, D], BF16, kind="ExternalInput")
    maska_d = nc.dram_tensor("maska", [128, 256], BF16, kind="ExternalInput")
    maskb_d = nc.dram_tensor("maskb", [128, 256], BF16, kind="ExternalInput")
    ident_d = nc.dram_tensor("ident", [128, 128], BF16, kind="ExternalInput")
    out_d = nc.dram_tensor("out", [S, D], F32, kind="ExternalOutput")

    with tile.TileContext(nc) as tc, ExitStack() as top:
        res = top.enter_context(tc.tile_pool(name="res", bufs=1))
        stats = top.enter_context(tc.tile_pool(name="stats", bufs=1))

        # ---- resident tiles -------------------------------------------------
        wq_sb = res.tile([128, 8, JC], F32R, tag="wq")
        wk_sb = res.tile([128, 8, JC], F32R, tag="wk")
        wv_sb = res.tile([128, 8, JC], F32R, tag="wv")
        nc.sync.dma_start(wq_sb, wqt_d[:, :].rearrange("(t p) j -> p t j", p=128))
        nc.sync.dma_start(wk_sb, wkt_d[:, :].rearrange("(t p) j -> p t j", p=128))
        nc.sync.dma_start(wv_sb, wvt_d[:, :].rearrange("(t p) j -> p t j", p=128))
        wo_sb = []
        for p2 in range(2):
            t = res.tile([128, D], BF16, tag=f"wo{p2}", name=f"wo{p2}")
            nc.sync.dma_start(t, wot_d[128 * p2 : 128 * (p2 + 1), :])
            wo_sb.append(t)
        maska = res.tile([128, 256], BF16, tag="maska")
        maskb = res.tile([128, 256], BF16, tag="maskb")
        ident = res.tile([128, 128], BF16, tag="ident")
        nc.sync.dma_start(maska, maska_d[:, :])
        nc.sync.dma_start(maskb, maskb_d[:, :])
        nc.sync.dma_start(ident, ident_d[:, :])

        # projected tensors (resident through attention), segmented 512-wide
        qseg = [[res.tile([128, 512], F32R, tag=f"qts{i}{c}", name=f"qts{i}{c}")
                 for c in range(4)] for i in range(2)]
        kseg = [[res.tile([128, 512], F32R, tag=f"kts{i}{c}", name=f"kts{i}{c}")
                 for c in range(4)] for i in range(2)]
        vu = [res.tile([128, JC], BF16, tag=f"vu{i}", name=f"vu{i}") for i in range(NQT)]
        ctxseg = [[res.tile([128, 512], BF16, tag=f"ctx{i}{c}", name=f"ctx{i}{c}")
                   for c in range(4)] for i in range(2)]

        def _one_pass(_rep):
            # single fused stage: projections interleaved with attention
            with ExitStack() as stage_c:
                xpool = stage_c.enter_context(tc.tile_pool(name=f"xt{_rep}", bufs=1))
                ppool = stage_c.enter_context(tc.tile_pool(name=f"pp{_rep}", bufs=1))
                ptp = stage_c.enter_context(tc.tile_pool(name=f"ptp{_rep}", bufs=1))
                obp = stage_c.enter_context(tc.tile_pool(name=f"obp{_rep}", bufs=1))
                pss_p = stage_c.enter_context(
                    tc.tile_pool(name=f"pss{_rep}", bufs=1, space="PSUM"))
                pst_p = stage_c.enter_context(
                    tc.tile_pool(name=f"pst{_rep}", bufs=1, space="PSUM"))
                psc_p = stage_c.enter_context(
                    tc.tile_pool(name=f"psc{_rep}", bufs=1, space="PSUM"))
                pso_p = stage_c.enter_context(
                    tc.tile_pool(name=f"pso{_rep}", bufs=1, space="PSUM"))

                ncopy = 0  # ptsb copy engine round-robin

                def proj_chunk(ch):
                    # 256-col subloads so the first matmuls start ~3us in
                    for kind, xd in (("k", xtk_d), ("q", xtq_d), ("v", xtv_d)):
                        ps = pss_p.tile([128, 1024], F32, tag="pss",
                                        bufs=_PSS_BUFS, name=f"psj{kind}")
                        for half in range(2):
                            sl = slice(ch * 512 + 256 * half,
                                       ch * 512 + 256 * (half + 1))
                            xc = xpool.tile([128, 8, 256], F32R, tag="xc",
                                            bufs=(2 if _DMA_TR else 3), name="xc")
                            nc.sync.dma_start(
                                xc, xd[:, sl].rearrange("(t p) s -> p t s", p=128)
                            )
                            if kind in ("q", "k"):
                                wsb = wq_sb if kind == "q" else wk_sb
                                for jt in range(2):
                                    for dt in range(8):
                                        nc.tensor.matmul(
                                            ps[:, 512 * jt + 256 * half :
                                               512 * jt + 256 * (half + 1)],
                                            wsb[:, dt, 128 * jt : 128 * (jt + 1)],
                                            xc[:, dt, :],
                                            start=(dt == 0),
                                            stop=(dt == 7),
                                        )
                            else:
                                for st2 in range(2):
                                    st = 2 * half + st2
                                    for dt in range(8):
                                        nc.tensor.matmul(
                                            ps[:, 256 * st : 256 * (st + 1)],
                                            xc[:, dt, st2 * 128 : (st2 + 1) * 128],
                                            wv_sb[:, dt, :],
                                            start=(dt == 0),
                                            stop=(dt == 7),
                                        )
                        if kind in ("q", "k"):
                            dst = qseg if kind == "q" else kseg
                            # round 0: copy 256-halves separately so the first
                            # score matmuls can start before the full chunk
                            # lands
                            spans = ((0, 256), (256, 512)) if ch == 0 else ((0, 512),)
                            for jt in range(2):
                                for lo_, hi_ in spans:
                                    psl = ps[:, 512 * jt + lo_ : 512 * jt + hi_]
                                    dsl = dst[jt][ch][:, lo_:hi_]
                                    if (ch + jt) % 2 == 0:
                                        nc.vector.tensor_copy(dsl, psl)
                                    else:
                                        nc.scalar.copy(dsl, psl)
                        else:
                            for st in range(4):
                                psl = ps[:, 256 * st : 256 * (st + 1)]
                                if st % 2 == 0:
                                    nc.vector.tensor_copy(vu[4 * ch + st], psl)
                                else:
                                    nc.scalar.copy(vu[4 * ch + st], psl)

                def emit_scores(u, p):
                    ptiles = {}
                    for sq in range(4):
                            qi = 4 * u + sq
                            W = 128 * (qi + 1)
                            even = (qi % 2 == 0)
                            Wp = W + 128 if even else W
                            msk = maskb if even else maska
                            nreg = 2 if Wp > 1024 else 1
                            winoff = Wp - 256
                            for h in range(2):
                                hsl = slice(64 * h, 64 * (h + 1))
                                qstat = qseg[p][u][hsl, 128 * sq : 128 * sq + 128]
                                if qi <= 7:
                                    pt = ppool.tile([128, 1024], BF16, tag="P",
                                                    bufs=(8 if _DMA_TR else 16), name=f"P{p}{sq}{h}")
                                else:
                                    pt = ppool.tile([128, 2048], BF16, tag="P2",
                                                    bufs=(6 if _DMA_TR else 11), name=f"P{p}{sq}{h}")
                                stt = stats.tile([128, 8], F32, tag="st",
                                                 bufs=24, name="stt")
                                regions = []
                                for r in range(nreg):
                                    lo = 1024 * r
                                    hi = min(1024 * (r + 1), Wp)
                                    wr = hi - lo
                                    wv_ = min(W - lo, wr)  # valid (unpadded) width
                                    ps = pss_p.tile([128, 1024], F32, tag="pss",
                                                    bufs=_PSS_BUFS, name="pss")
                                    regions.append((ps, lo, wv_))
                                    a = lo
                                    end_plain = min(hi, winoff)
                                    while a < end_plain:
                                        w = 512 if end_plain - a >= 512 else end_plain - a
                                        nc.tensor.matmul(
                                            ps[:, a - lo : a - lo + w],
                                            qstat,
                                            kseg[p][a // 512][hsl, a % 512 : a % 512 + w],
                                            start=True, stop=True,
                                            tile_position=(64 * h, 0),
                                        )
                                        a += w
                                    if hi > winoff:
                                        wo = winoff - lo
                                        nc.tensor.matmul(
                                            ps[:, wo : wo + 256], ident, msk,
                                            start=True, stop=False,
                                            skip_group_check=True,
                                        )
                                        ko = winoff % 512
                                        nc.tensor.matmul(
                                            ps[:, wo : wo + 256],
                                            qstat,
                                            kseg[p][winoff // 512][hsl, ko : ko + 256],
                                            start=False, stop=True,
                                            tile_position=(64 * h, 0),
                                            skip_group_check=True,
                                        )
                                    nc.vector.reduce_max(
                                        out=stt[:, r : r + 1], in_=ps[:, 0:wv_],
                                        axis=AX, negate=True,
                                    )
                                if nreg == 2:
                                    nc.vector.tensor_tensor(
                                        stt[:, 2:3], stt[:, 0:1], stt[:, 1:2],
                                        op=MIN,
                                    )
                                    bc = 2
                                else:
                                    bc = 0
                                for r, (ps, lo, wr) in enumerate(regions):
                                    nc.scalar.activation(
                                        out=pt[:, lo : lo + wr],
                                        in_=ps[:, 0:wr],
                                        func=EXP,
                                        bias=stt[:, bc : bc + 1],
                                        scale=1.0,
                                        accum_out=stt[:, 4 + r : 5 + r],
                                    )
                                if nreg == 2:
                                    nc.gpsimd.tensor_add(
                                        stt[:, 6:7], stt[:, 4:5], stt[:, 5:6]
                                    )
                                    zc = 6
                                else:
                                    zc = 4
                                nc.vector.reciprocal(stt[:, 7:8], stt[:, zc : zc + 1])
                                if u == 3 and _DMA_TR:
                                    # widest tiles: normalize on ACT so ACT
                                    # program order covers the last write,
                                    # then ACT issues the XBAR transpose (one
                                    # instr, all k-chunks); the tail needs no
                                    # PE transpose or copy
                                    nc.scalar.activation(
                                        out=pt[:, 0:W], in_=pt[:, 0:W],
                                        func=COPY, scale=stt[:, 7:8],
                                    )
                                    ptT = ppool.tile([128, 16, 128], BF16,
                                                     tag="PT", bufs=16,
                                                     name=f"PT{p}{sq}{h}")
                                    nc.scalar.dma_start_transpose(
                                        ptT[:, :, :], pt[:, :]
                                    )
                                    ptiles[(h, sq)] = (pt, Wp, ptT)
                                else:
                                    nc.vector.tensor_scalar_mul(
                                        pt[:, 0:W], pt[:, 0:W], stt[:, 7:8]
                                    )
                                    ptiles[(h, sq)] = (pt, Wp, None)
                    return ptiles

                def emit_tail(u, p, ptiles):
                    # transposes + PV for this (pair, supertile)
                    nonlocal ncopy
                    psc = psc_p.tile([128, 512], F32, tag="psc", bufs=1,
                                     name=f"psc{p}{u}")
                    for t in range(4 * u + 4):
                        vstart = max(0, t - 4 * u)
                        if u == 3 and _DMA_TR:
                            # DMA-transposed path: PV reads ptT directly
                            for h in range(2):
                                for sq in range(vstart, 4):
                                    nc.tensor.matmul(
                                        psc[64 * h : 64 * (h + 1),
                                            128 * sq : 128 * (sq + 1)],
                                        vu[t][:, 64 * (2 * p + h) : 64 * (2 * p + h + 1)],
                                        ptiles[(h, sq)][2][:, t, :],
                                        start=(t == 0),
                                        stop=(t == 4 * u + 3),
                                        tile_position=(0, 64 * h),
                                        skip_group_check=True,
                                    )
                            continue
                        pstile = pst_p.tile([128, 1024], BF16, tag="pst",
                                            bufs=1, name="pst")
                        for h in range(2):
                            for sq in range(vstart, 4):
                                nc.tensor.transpose(
                                    pstile[:, 512 * h + 128 * sq : 512 * h + 128 * (sq + 1)],
                                    ptiles[(h, sq)][0][:, 128 * t : 128 * (t + 1)],
                                    ident,
                                )
                        ptsb = ptp.tile([128, 1024], BF16, tag="pt", bufs=(2 if _DMA_TR else 3),
                                        name="ptsb")
                        if vstart == 0:
                            slices = [slice(0, 1024)]
                        else:
                            slices = [
                                slice(512 * h + 128 * vstart, 512 * (h + 1))
                                for h in range(2)
                            ]
                        for wsl in slices:
                            if _PTSB_PAT[ncopy % len(_PTSB_PAT)] == 0:
                                nc.vector.tensor_copy(ptsb[:, wsl], pstile[:, wsl])
                            else:
                                nc.scalar.copy(ptsb[:, wsl], pstile[:, wsl])
                            ncopy += 1
                        for h in range(2):
                            csl = slice(128 * vstart, 512)
                            nc.tensor.matmul(
                                psc[64 * h : 64 * (h + 1), csl],
                                vu[t][:, 64 * (2 * p + h) : 64 * (2 * p + h + 1)],
                                ptsb[:, 512 * h :][:, csl],
                                start=(t == 0),
                                stop=(t == 4 * u + 3),
                                tile_position=(0, 64 * h),
                                skip_group_check=True,
                            )
                    if u % 2 == 0:
                        nc.vector.tensor_copy(ctxseg[p][u], psc)
                    else:
                        nc.scalar.copy(ctxseg[p][u], psc)
                    if p != 1:
                        return
                    # output projection for the four finished s-tiles
                    for st_ in range(4 * u, 4 * u + 4):
                        ssl = slice(128 * st_, 128 * (st_ + 1))
                        csl_ = slice(128 * (st_ % 4), 128 * (st_ % 4) + 128)
                        for oc in range(2):
                            osl = slice(512 * oc, 512 * (oc + 1))
                            pso = pso_p.tile([128, 512], F32, tag="pso", bufs=1,
                                             name="pso")
                            nc.tensor.matmul(pso, ctxseg[0][u][:, csl_],
                                             wo_sb[0][:, osl],
                                             start=True, stop=False)
                            nc.tensor.matmul(pso, ctxseg[1][u][:, csl_],
                                             wo_sb[1][:, osl],
                                             start=False, stop=True)
                            osb = obp.tile([128, 512], F32, tag="ob", bufs=(1 if _DMA_TR else 3),
                                           name="osb")
                            if oc == 0:
                                nc.vector.tensor_copy(osb, pso)
                            else:
                                nc.scalar.copy(osb, pso)
                            nc.gpsimd.dma_start(out_d[ssl, osl], osb)

                # software pipeline: proj chunk u feeds attention round u;
                # scores of group g+1 overlap tail of group g
                pending = None
                for u in range(NU4):
                    proj_chunk(u)
                    for p in range(2):
                        if _DMA_TR and (u, p) == (3, 1) and pending is not None:
                            # PT pool holds one group's transposed tiles; flush
                            # the pending tail before its buffers are reused
                            emit_tail(*pending)
                            pending = None
                        ptiles = emit_scores(u, p)
                        if pending is not None:
                            emit_tail(*pending)
                        pending = (u, p, ptiles)
                emit_tail(*pending)

        for _rep in range(reps):
            if _rep:
                tc.strict_bb_all_engine_barrier()
            _one_pass(_rep)

    nc.compile()
    return nc


def _get_nc(reps=1):
    key = ("nc", reps)
    if key not in _cached:
        _cached[key] = _build_nc(reps)
    return _cached[key]


def _fp22(a):
    """Truncate fp32 to fp22 (e8m13) as the PE's float32r datapath does."""
    a = np.ascontiguousarray(a, dtype=np.float32)
    a.view(np.uint32)[...] &= np.uint32(0xFFFFFC00)
    return a


def _host_inputs(query, key, value, Wq, Wk, Wv, Wo):
    """Build the 8 per-core input dicts (host-side transposes/slices)."""
    f32 = np.float32
    xt = {}
    for b in range(B):
        xt[("q", b)] = _fp22(query[b].T)
        xt[("k", b)] = _fp22(key[b].T)
        xt[("v", b)] = _fp22(value[b].T)
    import ml_dtypes

    q_ar = np.arange(128)[:, None]
    j_ar = np.arange(128)[None, :]
    tri = np.where(j_ar <= q_ar, 0.0, _MASKVAL).astype(f32)
    maska = np.concatenate([np.zeros((128, 128), f32), tri], axis=1)
    maskb = np.concatenate([tri, np.full((128, 128), _MASKVAL, f32)], axis=1)
    maska = maska.astype(ml_dtypes.bfloat16)
    maskb = maskb.astype(ml_dtypes.bfloat16)
    ident = np.eye(128).astype(ml_dtypes.bfloat16)
    in_maps = []
    for c in range(8):
        b, g = c // 4, c % 4
        jsl = slice(JC * g, JC * (g + 1))
        in_maps.append(
            {
                "xtq": xt[("q", b)],
                "xtk": xt[("k", b)],
                "xtv": xt[("v", b)],
                "wqt": _fp22(_SCALE * Wq[jsl, :].T),
                "wkt": _fp22(Wk[jsl, :].T),
                "wvt": _fp22(Wv[jsl, :].T),
                "wot": np.ascontiguousarray(Wo[:, jsl].T).astype(ml_dtypes.bfloat16),
                "maska": maska,
                "maskb": maskb,
                "ident": ident,
            }
        )
    return in_maps


def _numpy_fallback(query, key, value, mask, Wq, Wk, Wv, Wo):
    """Exact (chunked) numpy path for non-causal masks."""
    out = np.empty((B, S, D), dtype=np.float32)
    q = (query @ Wq.T).reshape(B, S, H, DK).transpose(0, 2, 1, 3)
    k = (key @ Wk.T).reshape(B, S, H, DK).transpose(0, 2, 1, 3)
    v = (value @ Wv.T).reshape(B, S, H, DK).transpose(0, 2, 1, 3)
    for b in range(B):
        ctx = np.empty((H, S, DK), dtype=np.float32)
        mb = mask[b] == 0
        for h in range(H):
            s = (q[b, h] @ k[b, h].T) * _SCALE
            s[mb] = np.finfo(np.float32).min
            s -= s.max(axis=1, keepdims=True)
            np.exp(s, out=s)
            s /= s.sum(axis=1, keepdims=True)
            ctx[h] = s @ v[b, h]
        out[b] = ctx.transpose(1, 0, 2).reshape(S, D) @ Wo.T
    return out


def kernel(query, key, value, mask, Wq, Wk, Wv, Wo):
    query = np.asarray(query, dtype=np.float32)
    key = np.asarray(key, dtype=np.float32)
    value = np.asarray(value, dtype=np.float32)
    mask = np.asarray(mask)
    Wq, Wk, Wv, Wo = (np.asarray(w, dtype=np.float32) for w in (Wq, Wk, Wv, Wo))

    tril = np.tril(np.ones((S, S), dtype=mask.dtype))
    if not all(np.array_equal(mask[b], tril) for b in range(B)):
        return _numpy_fallback(query, key, value, mask, Wq, Wk, Wv, Wo)

    from concourse.bass_utils import run_bass_kernel_spmd

    nc = _get_nc()
    in_maps = _host_inputs(query, key, value, Wq, Wk, Wv, Wo)
    res = run_bass_kernel_spmd(nc, in_maps, core_ids=list(range(8)))
    outs = [r["out"] for r in res.results]
    full = np.empty((B, S, D), dtype=np.float32)
    for b in range(B):
        full[b] = outs[4 * b] + outs[4 * b + 1] + outs[4 * b + 2] + outs[4 * b + 3]
    return full
